# revision 5
# baseline (speedup 1.0000x reference)
"""Trainium2 Bass kernel for nn_AdditiveAttention (B=32, NQ=1, NK=4096, D=512, H=256).

Data-parallel over 8 NeuronCores: each core owns 4 batches. Per core:
  kprojT[h, t] = sum_d W_k[d, h] * keys[b, t, d]      (PE, f32r, W_k stationary)
  featT        = tanh(kprojT + qproj_b)               (ACT, bias fused, bf16 out)
  scores[t]    = sum_h w_v[h] * featT[h, t]           (PE matvec, bf16)
  out[b, t]    = softmax_t(scores) * values[b, t]     (DVE/ACT, exp bias=-max,
                                                       accum_out=sum fused)

The keys shard is handed to the device pre-transposed ([4, 512, 4096]) so the
contraction dim lands on SBUF partitions; HBM traffic is unchanged (256 MiB of
keys dominate -> memory-bound at ~358 GB/s/core).
"""

import numpy as np

N_CORES = 8
B, NQ, NK, D, H = 32, 1, 4096, 512, 256
B_LOC = B // N_CORES  # 4 batches per core
KT = D // 128         # 4 contraction tiles
HT = H // 128         # 2 hidden tiles
TOKC = 512            # matmul moving-dim chunk (= one PSUM bank of f32)
THALF = 2048          # keys DMA tile width (tokens) -> 1 MiB per DMA
NHALF = NK // THALF   # 2
NCH = NK // TOKC      # 8 chunks per batch
CPH = THALF // TOKC   # 4 chunks per half


def _install_profile_hook():
    """Make trace=True / BASS_TRACE=1 usable when the image's antenv lacks
    axon_hooks (degrades silently if anything is missing)."""
    try:
        from antenv import axon_hooks  # noqa: F401
        return
    except ImportError:
        pass
    try:
        import sys
        import types

        import antenv
        from trn_agent_boot.trn_boot import _ntff_profile_via_ctypes

        mod = types.ModuleType("antenv.axon_hooks")
        mod._h = None
        mod.set_axon_ntff_profile_hook = lambda h: setattr(mod, "_h", h)
        mod.get_axon_ntff_profile_hook = lambda: mod._h
        antenv.axon_hooks = mod
        sys.modules["antenv.axon_hooks"] = mod
        mod._h = _ntff_profile_via_ctypes("/opt/axon/libaxon_pjrt.so")
    except Exception:
        pass


def build_nc():
    import concourse.tile as tile
    from concourse import bacc, mybir

    f32 = mybir.dt.float32
    f32r = mybir.dt.float32r
    bf16 = mybir.dt.bfloat16
    Act = mybir.ActivationFunctionType
    AX = mybir.AxisListType.X

    nc = bacc.Bacc("TRN2", target_bir_lowering=False, debug=False,
                   num_devices=N_CORES)

    keysT_ext = nc.dram_tensor("keysT", [B_LOC, D, NK], f32r, kind="ExternalInput")
    qT_ext = nc.dram_tensor("queriesT", [D, B_LOC], f32, kind="ExternalInput")
    vals_ext = nc.dram_tensor("vals", [B_LOC, NK], f32, kind="ExternalInput")
    wk_ext = nc.dram_tensor("wk", [D, H], f32r, kind="ExternalInput")
    wq_ext = nc.dram_tensor("wq", [D, H], f32, kind="ExternalInput")
    wv_ext = nc.dram_tensor("wv", [128, HT], f32, kind="ExternalInput")
    out_ext = nc.dram_tensor("out", [B_LOC, NK], f32, kind="ExternalOutput")

    with tile.TileContext(nc) as tc:
        with (
            tc.tile_pool(name="keys", bufs=12) as keys_pool,
            tc.tile_pool(name="feat", bufs=6) as feat_pool,
            tc.tile_pool(name="static", bufs=1) as st,
            tc.tile_pool(name="kp", bufs=4, space="PSUM") as kp_pool,
            tc.tile_pool(name="sc", bufs=2, space="PSUM") as sc_pool,
        ):
            # ---- one-time loads ----
            wk_sb = st.tile([128, KT, H], f32r, tag="wk")
            wq_sb = st.tile([128, KT, H], f32, tag="wq")
            qin_sb = st.tile([128, KT, B_LOC], f32, tag="qin")
            for k in range(KT):
                nc.sync.dma_start(wk_sb[:, k, :], wk_ext[k * 128:(k + 1) * 128, :])
                nc.sync.dma_start(wq_sb[:, k, :], wq_ext[k * 128:(k + 1) * 128, :])
                nc.sync.dma_start(qin_sb[:, k, :], qT_ext[k * 128:(k + 1) * 128, :])
            wv_sb = st.tile([128, HT], f32, tag="wv")
            nc.sync.dma_start(wv_sb[:], wv_ext.ap())
            wv_bf = st.tile([128, HT], bf16, tag="wvbf")
            nc.vector.tensor_copy(wv_bf[:], wv_sb[:])
            # per-batch softmax rows live at partition 32*b (engine ops need
            # 32-aligned base partitions)
            vals_sb = st.tile([128, NK], f32, tag="vals")
            for b in range(B_LOC):
                nc.sync.dma_start(vals_sb[32 * b:32 * b + 1, :],
                                  vals_ext[b:b + 1, :])

            # ---- qproj (f32, exact): qbias[h][:, b] = (queries @ W_q)^T ----
            qbias_sb = st.tile([128, HT, B_LOC], f32, tag="qbias")
            for h in range(HT):
                qp = sc_pool.tile([128, B_LOC], f32, tag="qp")
                for k in range(KT):
                    nc.tensor.matmul(
                        qp[:],
                        wq_sb[:, k, h * 128:(h + 1) * 128],
                        qin_sb[:, k, :],
                        start=(k == 0), stop=(k == KT - 1),
                    )
                nc.vector.tensor_copy(qbias_sb[:, h, :], qp[:])

            # ---- per-batch softmax state (row 32*b per batch) ----
            scores_sb = st.tile([128, NK], f32, tag="scores")
            pmax_sb = st.tile([128, NCH], f32, tag="pmax")
            negmax_sb = st.tile([128, 1], f32, tag="negmax")
            ssum_sb = st.tile([128, 1], f32, tag="ssum")
            recip_sb = st.tile([128, 1], f32, tag="recip")

            for b in range(B_LOC):
                r = 32 * b
                # keys shard for this batch, [128 d, THALF tok] tiles
                kt_tiles = {}
                for half in range(NHALF):
                    for k in range(KT):
                        t = keys_pool.tile([128, THALF], f32r, tag="kt")
                        nc.sync.dma_start(
                            t[:],
                            keysT_ext[b, k * 128:(k + 1) * 128,
                                      half * THALF:(half + 1) * THALF],
                        )
                        kt_tiles[(half, k)] = t

                for c in range(NCH):
                    half, off = c // CPH, (c % CPH) * TOKC
                    fts = []
                    for h in range(HT):
                        ps = kp_pool.tile([128, TOKC], f32, tag="ps")
                        for k in range(KT):
                            nc.tensor.matmul(
                                ps[:],
                                wk_sb[:, k, h * 128:(h + 1) * 128],
                                kt_tiles[(half, k)][:, off:off + TOKC],
                                start=(k == 0), stop=(k == KT - 1),
                            )
                        ft = feat_pool.tile([128, TOKC], bf16, tag="ft")
                        nc.scalar.activation(ft[:], ps[:], Act.Tanh,
                                             bias=qbias_sb[:, h, b:b + 1])
                        fts.append(ft)
                    sc = sc_pool.tile([128, TOKC], f32, tag="sc")
                    for h in range(HT):
                        nc.tensor.matmul(sc[r:r + 1, :], wv_bf[:, h:h + 1],
                                         fts[h][:], start=(h == 0),
                                         stop=(h == HT - 1),
                                         tile_position=(0, r))
                    cs = c * TOKC
                    nc.vector.tensor_copy(scores_sb[r:r + 1, cs:cs + TOKC],
                                          sc[r:r + 1, :])
                    nc.vector.reduce_max(pmax_sb[r:r + 1, c:c + 1],
                                         sc[r:r + 1, :], axis=AX)

                # softmax * values for this batch (overlaps next batch's compute)
                nc.vector.reduce_max(negmax_sb[r:r + 1, :], pmax_sb[r:r + 1, :],
                                     axis=AX, negate=True)
                nc.scalar.activation(scores_sb[r:r + 1, :], scores_sb[r:r + 1, :],
                                     Act.Exp, bias=negmax_sb[r:r + 1, :],
                                     accum_out=ssum_sb[r:r + 1, :])
                nc.vector.tensor_mul(scores_sb[r:r + 1, :], scores_sb[r:r + 1, :],
                                     vals_sb[r:r + 1, :])
                nc.vector.reciprocal(recip_sb[r:r + 1, :], ssum_sb[r:r + 1, :])
                nc.vector.tensor_scalar_mul(scores_sb[r:r + 1, :],
                                            scores_sb[r:r + 1, :],
                                            recip_sb[r:r + 1, :])
                nc.sync.dma_start(out_ext[b:b + 1, :], scores_sb[r:r + 1, :])

    nc.compile()
    return nc


def shard_inputs(queries, keys, values, W_q, W_k, w_v):
    queries = np.asarray(queries, np.float32)
    keys = np.asarray(keys, np.float32)
    values = np.asarray(values, np.float32)
    W_q = np.asarray(W_q, np.float32)
    W_k = np.asarray(W_k, np.float32)
    w_v = np.asarray(w_v, np.float32)

    wv2 = np.ascontiguousarray(w_v.reshape(HT, 128).T)  # [128, HT]
    in_maps = []
    for i in range(N_CORES):
        b0, b1 = i * B_LOC, (i + 1) * B_LOC
        in_maps.append({
            "keysT": np.ascontiguousarray(keys[b0:b1].transpose(0, 2, 1)),
            "queriesT": np.ascontiguousarray(queries[b0:b1, 0, :].T),
            "vals": np.ascontiguousarray(values[b0:b1, :, 0]),
            "wk": W_k, "wq": W_q, "wv": wv2,
        })
    return in_maps


_NC_CACHE = {}


def run(in_maps, trace=False, tmpdir=None):
    from concourse.bass_utils import run_bass_kernel_spmd

    _install_profile_hook()
    if trace:
        # no artifact bucket inside the container; keep traces local
        import concourse.bass_utils as bu
        bu.upload_artifacts = lambda d: "local://" + d
    if "nc" not in _NC_CACHE:
        _NC_CACHE["nc"] = build_nc()
    nc = _NC_CACHE["nc"]
    return run_bass_kernel_spmd(nc, in_maps, core_ids=list(range(N_CORES)),
                                trace=trace, tmpdir=tmpdir)


def kernel(queries, keys, values, W_q, W_k, w_v):
    in_maps = shard_inputs(queries, keys, values, W_q, W_k, w_v)
    res = run(in_maps)
    return np.concatenate([res.results[i]["out"] for i in range(N_CORES)], axis=0)


# revision 8
# speedup vs baseline: 1.2295x; 1.2295x over previous
"""Trainium2 Bass kernel for nn_AdditiveAttention (B=32, NQ=1, NK=4096, D=512, H=256).

Data-parallel over 8 NeuronCores: each core owns 4 batches. Per core:
  kprojT[h, t] = sum_d W_k[d, h] * keys[b, t, d]      (PE, bf16, W_k stationary)
  featT        = tanh(kprojT + qproj_b)               (ACT, bias fused, bf16 out)
  scores[t]    = sum_h w_v[h] * featT[h, t]           (PE matvec, bf16)
  out[b, t]    = softmax_t(scores) * values[b, t]     (exp straight from PSUM with
                                                       fused partial sums; scores
                                                       are O(4) so no max-subtract;
                                                       (e*1/sum)*values fused on
                                                       GpSimd)

The keys shard is handed to the device pre-transposed ([4, 512, 4096]) so the
contraction dim lands on SBUF partitions, and is cast f32->bf16 inside the
HBM->SBUF DMA (SWDGE). HBM traffic is unchanged (256 MiB of f32 keys dominate
-> memory-bound at ~358 GB/s/core).
"""

import numpy as np

N_CORES = 8
B, NQ, NK, D, H = 32, 1, 4096, 512, 256
B_LOC = B // N_CORES  # 4 batches per core
KT = D // 128         # 4 contraction tiles
HT = H // 128         # 2 hidden tiles
TOKC = 512            # matmul moving-dim chunk (= one PSUM bank of f32)
THALF = 2048          # keys DMA tile width (tokens) -> 1 MiB f32 per DMA
NHALF = NK // THALF   # 2
NCH = NK // TOKC      # 8 chunks per batch
CPH = THALF // TOKC   # 4 chunks per half


def _install_profile_hook():
    """Make trace=True / BASS_TRACE=1 usable when the image's antenv lacks
    axon_hooks (degrades silently if anything is missing)."""
    try:
        from antenv import axon_hooks  # noqa: F401
        return
    except ImportError:
        pass
    try:
        import sys
        import types

        import antenv
        from trn_agent_boot.trn_boot import _ntff_profile_via_ctypes

        mod = types.ModuleType("antenv.axon_hooks")
        mod._h = None
        mod.set_axon_ntff_profile_hook = lambda h: setattr(mod, "_h", h)
        mod.get_axon_ntff_profile_hook = lambda: mod._h
        antenv.axon_hooks = mod
        sys.modules["antenv.axon_hooks"] = mod
        mod._h = _ntff_profile_via_ctypes("/opt/axon/libaxon_pjrt.so")
    except Exception:
        pass


def build_nc():
    import concourse.tile as tile
    from concourse import bacc, mybir

    f32 = mybir.dt.float32
    bf16 = mybir.dt.bfloat16
    Act = mybir.ActivationFunctionType
    AX = mybir.AxisListType.X

    nc = bacc.Bacc("TRN2", target_bir_lowering=False, debug=False,
                   num_devices=N_CORES)

    keysT_ext = nc.dram_tensor("keysT", [B_LOC, D, NK], f32, kind="ExternalInput")
    qT_ext = nc.dram_tensor("queriesT", [D, B_LOC], f32, kind="ExternalInput")
    vals_ext = nc.dram_tensor("vals", [B_LOC, NK], f32, kind="ExternalInput")
    wk_ext = nc.dram_tensor("wk", [D, H], f32, kind="ExternalInput")
    wq_ext = nc.dram_tensor("wq", [D, H], f32, kind="ExternalInput")
    wv_ext = nc.dram_tensor("wv", [128, HT], f32, kind="ExternalInput")
    out_ext = nc.dram_tensor("out", [B_LOC, NK], f32, kind="ExternalOutput")

    with tile.TileContext(nc) as tc:
        with (
            tc.tile_pool(name="keys", bufs=16) as keys_pool,
            tc.tile_pool(name="feat", bufs=6) as feat_pool,
            tc.tile_pool(name="static", bufs=1) as st,
            tc.tile_pool(name="kp", bufs=4, space="PSUM") as kp_pool,
            tc.tile_pool(name="sc", bufs=2, space="PSUM") as sc_pool,
        ):
            # ---- one-time loads (weights cast to bf16 inside the DMA) ----
            wk_bf = st.tile([128, KT, H], bf16, tag="wk")
            wq_sb = st.tile([128, KT, H], f32, tag="wq")
            qin_sb = st.tile([128, KT, B_LOC], f32, tag="qin")
            for k in range(KT):
                nc.gpsimd.dma_start(wk_bf[:, k, :], wk_ext[k * 128:(k + 1) * 128, :])
                nc.sync.dma_start(wq_sb[:, k, :], wq_ext[k * 128:(k + 1) * 128, :])
                nc.sync.dma_start(qin_sb[:, k, :], qT_ext[k * 128:(k + 1) * 128, :])
            wv_bf = st.tile([128, HT], bf16, tag="wvbf")
            nc.gpsimd.dma_start(wv_bf[:], wv_ext.ap())
            # per-batch softmax rows live at partition 32*b (engine ops need
            # 32-aligned base partitions)
            vals_sb = st.tile([128, NK], f32, tag="vals")
            for b in range(B_LOC):
                nc.sync.dma_start(vals_sb[32 * b:32 * b + 1, :],
                                  vals_ext[b:b + 1, :])

            # ---- qproj (f32, exact): qbias[h][:, b] = (queries @ W_q)^T ----
            qbias_sb = st.tile([128, HT, B_LOC], f32, tag="qbias")
            for h in range(HT):
                qp = sc_pool.tile([128, B_LOC], f32, tag="qp")
                for k in range(KT):
                    nc.tensor.matmul(
                        qp[:],
                        wq_sb[:, k, h * 128:(h + 1) * 128],
                        qin_sb[:, k, :],
                        start=(k == 0), stop=(k == KT - 1),
                    )
                nc.vector.tensor_copy(qbias_sb[:, h, :], qp[:])

            # ---- per-batch softmax state (row 32*b per batch) ----
            esc_sb = st.tile([128, NK], f32, tag="esc")       # exp(scores)
            psum_sb = st.tile([128, NCH], f32, tag="psums")   # partial exp sums
            ssum_sb = st.tile([128, 1], f32, tag="ssum")
            recip_sb = st.tile([128, 1], f32, tag="recip")

            for b in range(B_LOC):
                r = 32 * b
                # keys shard for this batch: [128 d, THALF tok] bf16 tiles,
                # f32->bf16 cast inside the DMA (SWDGE)
                kt_tiles = {}
                for half in range(NHALF):
                    for k in range(KT):
                        t = keys_pool.tile([128, THALF], bf16, tag="kt")
                        nc.gpsimd.dma_start(
                            t[:],
                            keysT_ext[b, k * 128:(k + 1) * 128,
                                      half * THALF:(half + 1) * THALF],
                        )
                        kt_tiles[(half, k)] = t

                for c in range(NCH):
                    half, off = c // CPH, (c % CPH) * TOKC
                    fts = []
                    for h in range(HT):
                        ps = kp_pool.tile([128, TOKC], f32, tag="ps")
                        for k in range(KT):
                            nc.tensor.matmul(
                                ps[:],
                                wk_bf[:, k, h * 128:(h + 1) * 128],
                                kt_tiles[(half, k)][:, off:off + TOKC],
                                start=(k == 0), stop=(k == KT - 1),
                            )
                        ft = feat_pool.tile([128, TOKC], bf16, tag="ft")
                        nc.scalar.activation(ft[:], ps[:], Act.Tanh,
                                             bias=qbias_sb[:, h, b:b + 1])
                        fts.append(ft)
                    sc = sc_pool.tile([128, TOKC], f32, tag="sc")
                    for h in range(HT):
                        nc.tensor.matmul(sc[r:r + 1, :], wv_bf[:, h:h + 1],
                                         fts[h][:], start=(h == 0),
                                         stop=(h == HT - 1),
                                         tile_position=(0, r))
                    # exp straight off PSUM; partial sum fused into the same op
                    cs = c * TOKC
                    nc.scalar.activation(esc_sb[r:r + 1, cs:cs + TOKC],
                                         sc[r:r + 1, :], Act.Exp,
                                         accum_out=psum_sb[r:r + 1, c:c + 1])

                # softmax denominator + (e * 1/sum) * values, fused on GpSimd
                nc.vector.reduce_sum(ssum_sb[r:r + 1, :], psum_sb[r:r + 1, :],
                                     axis=AX)
                nc.vector.reciprocal(recip_sb[r:r + 1, :], ssum_sb[r:r + 1, :])
                nc.gpsimd.tensor_mul(esc_sb[r:r + 1, :], esc_sb[r:r + 1, :],
                                     vals_sb[r:r + 1, :])
                nc.vector.tensor_scalar_mul(esc_sb[r:r + 1, :],
                                            esc_sb[r:r + 1, :],
                                            recip_sb[r:r + 1, :])
                nc.sync.dma_start(out_ext[b:b + 1, :], esc_sb[r:r + 1, :])

    nc.compile()
    return nc


def shard_inputs(queries, keys, values, W_q, W_k, w_v):
    queries = np.asarray(queries, np.float32)
    keys = np.asarray(keys, np.float32)
    values = np.asarray(values, np.float32)
    W_q = np.asarray(W_q, np.float32)
    W_k = np.asarray(W_k, np.float32)
    w_v = np.asarray(w_v, np.float32)

    wv2 = np.ascontiguousarray(w_v.reshape(HT, 128).T)  # [128, HT]
    in_maps = []
    for i in range(N_CORES):
        b0, b1 = i * B_LOC, (i + 1) * B_LOC
        in_maps.append({
            "keysT": np.ascontiguousarray(keys[b0:b1].transpose(0, 2, 1)),
            "queriesT": np.ascontiguousarray(queries[b0:b1, 0, :].T),
            "vals": np.ascontiguousarray(values[b0:b1, :, 0]),
            "wk": W_k, "wq": W_q, "wv": wv2,
        })
    return in_maps


_NC_CACHE = {}


def run(in_maps, trace=False, tmpdir=None):
    from concourse.bass_utils import run_bass_kernel_spmd

    _install_profile_hook()
    if trace:
        # no artifact bucket inside the container; keep traces local
        import concourse.bass_utils as bu
        bu.upload_artifacts = lambda d: "local://" + d
    if "nc" not in _NC_CACHE:
        _NC_CACHE["nc"] = build_nc()
    nc = _NC_CACHE["nc"]
    return run_bass_kernel_spmd(nc, in_maps, core_ids=list(range(N_CORES)),
                                trace=trace, tmpdir=tmpdir)


def kernel(queries, keys, values, W_q, W_k, w_v):
    in_maps = shard_inputs(queries, keys, values, W_q, W_k, w_v)
    res = run(in_maps)
    return np.concatenate([res.results[i]["out"] for i in range(N_CORES)], axis=0)


# revision 9
# speedup vs baseline: 1.3840x; 1.1257x over previous
"""Trainium2 Bass kernel for nn_AdditiveAttention (B=32, NQ=1, NK=4096, D=512, H=256).

Data-parallel over 8 NeuronCores: each core owns 4 batches. Per core:
  kprojT[h, t] = sum_d W_k[d, h] * keys[b, t, d]      (PE, bf16, W_k stationary)
  featT        = tanh(kprojT + qproj_b)               (ACT, bias fused, bf16 out)
  scores[t]    = sum_h w_v[h] * featT[h, t]           (PE matvec, bf16)
  out[b, t]    = softmax_t(scores) * values[b, t]     (exp straight from PSUM with
                                                       fused partial sums; scores
                                                       are O(4) so no max-subtract;
                                                       (e*1/sum)*values fused on
                                                       GpSimd)

The keys shard is handed to the device pre-transposed ([4, 512, 4096]) so the
contraction dim lands on SBUF partitions, and is cast f32->bf16 inside the
HBM->SBUF DMA (SWDGE). HBM traffic is unchanged (256 MiB of f32 keys dominate
-> memory-bound at ~358 GB/s/core).
"""

import numpy as np

N_CORES = 8
B, NQ, NK, D, H = 32, 1, 4096, 512, 256
B_LOC = B // N_CORES  # 4 batches per core
KT = D // 128         # 4 contraction tiles
HT = H // 128         # 2 hidden tiles
TOKC = 512            # matmul moving-dim chunk (= one PSUM bank of f32)
THALF = 2048          # keys DMA tile width (tokens) -> 1 MiB f32 per DMA
NHALF = NK // THALF   # 2
NCH = NK // TOKC      # 8 chunks per batch
CPH = THALF // TOKC   # 4 chunks per half


def _install_profile_hook():
    """Make trace=True / BASS_TRACE=1 usable when the image's antenv lacks
    axon_hooks (degrades silently if anything is missing)."""
    try:
        from antenv import axon_hooks  # noqa: F401
        return
    except ImportError:
        pass
    try:
        import sys
        import types

        import antenv
        from trn_agent_boot.trn_boot import _ntff_profile_via_ctypes

        mod = types.ModuleType("antenv.axon_hooks")
        mod._h = None
        mod.set_axon_ntff_profile_hook = lambda h: setattr(mod, "_h", h)
        mod.get_axon_ntff_profile_hook = lambda: mod._h
        antenv.axon_hooks = mod
        sys.modules["antenv.axon_hooks"] = mod
        mod._h = _ntff_profile_via_ctypes("/opt/axon/libaxon_pjrt.so")
    except Exception:
        pass


def build_nc():
    import concourse.tile as tile
    from concourse import bacc, mybir

    f32 = mybir.dt.float32
    bf16 = mybir.dt.bfloat16
    Act = mybir.ActivationFunctionType
    AX = mybir.AxisListType.X

    nc = bacc.Bacc("TRN2", target_bir_lowering=False, debug=False,
                   num_devices=N_CORES)

    keysT_ext = nc.dram_tensor("keysT", [B_LOC, D, NK], f32, kind="ExternalInput")
    qT_ext = nc.dram_tensor("queriesT", [D, B_LOC], f32, kind="ExternalInput")
    vals_ext = nc.dram_tensor("vals", [B_LOC, NK], f32, kind="ExternalInput")
    wk_ext = nc.dram_tensor("wk", [D, H], f32, kind="ExternalInput")
    wq_ext = nc.dram_tensor("wq", [D, H], f32, kind="ExternalInput")
    wv_ext = nc.dram_tensor("wv", [128, HT], f32, kind="ExternalInput")
    out_ext = nc.dram_tensor("out", [B_LOC, NK], f32, kind="ExternalOutput")

    with tile.TileContext(nc) as tc:
        with (
            tc.tile_pool(name="keys", bufs=24) as keys_pool,
            tc.tile_pool(name="feat", bufs=6) as feat_pool,
            tc.tile_pool(name="static", bufs=1) as st,
            tc.tile_pool(name="kp", bufs=4, space="PSUM") as kp_pool,
            tc.tile_pool(name="sc", bufs=2, space="PSUM") as sc_pool,
        ):
            # ---- one-time loads (weights cast to bf16 inside the DMA) ----
            wk_bf = st.tile([128, KT, H], bf16, tag="wk")
            wq_sb = st.tile([128, KT, H], f32, tag="wq")
            qin_sb = st.tile([128, KT, B_LOC], f32, tag="qin")
            for k in range(KT):
                nc.gpsimd.dma_start(wk_bf[:, k, :], wk_ext[k * 128:(k + 1) * 128, :])
                nc.sync.dma_start(wq_sb[:, k, :], wq_ext[k * 128:(k + 1) * 128, :])
                nc.sync.dma_start(qin_sb[:, k, :], qT_ext[k * 128:(k + 1) * 128, :])
            wv_bf = st.tile([128, HT], bf16, tag="wvbf")
            nc.gpsimd.dma_start(wv_bf[:], wv_ext.ap())
            # per-batch softmax rows live at partition 32*b (engine ops need
            # 32-aligned base partitions)
            vals_sb = st.tile([128, NK], f32, tag="vals")
            for b in range(B_LOC):
                nc.sync.dma_start(vals_sb[32 * b:32 * b + 1, :],
                                  vals_ext[b:b + 1, :])

            # ---- qproj (f32, exact): qbias[h][:, b] = (queries @ W_q)^T ----
            qbias_sb = st.tile([128, HT, B_LOC], f32, tag="qbias")
            for h in range(HT):
                qp = sc_pool.tile([128, B_LOC], f32, tag="qp")
                for k in range(KT):
                    nc.tensor.matmul(
                        qp[:],
                        wq_sb[:, k, h * 128:(h + 1) * 128],
                        qin_sb[:, k, :],
                        start=(k == 0), stop=(k == KT - 1),
                    )
                nc.vector.tensor_copy(qbias_sb[:, h, :], qp[:])

            # ---- per-batch softmax state (row 32*b per batch) ----
            esc_sb = st.tile([128, NK], f32, tag="esc")       # exp(scores)
            psum_sb = st.tile([128, NCH], f32, tag="psums")   # partial exp sums
            ssum_sb = st.tile([128, 1], f32, tag="ssum")
            recip_sb = st.tile([128, 1], f32, tag="recip")

            for b in range(B_LOC):
                r = 32 * b
                # keys shard for this batch: [128 d, THALF tok] bf16 tiles,
                # f32->bf16 cast inside the DMA (SWDGE)
                kt_tiles = {}
                for half in range(NHALF):
                    for k in range(KT):
                        t = keys_pool.tile([128, THALF], bf16, tag="kt")
                        nc.gpsimd.dma_start(
                            t[:],
                            keysT_ext[b, k * 128:(k + 1) * 128,
                                      half * THALF:(half + 1) * THALF],
                        )
                        kt_tiles[(half, k)] = t

                for c in range(NCH):
                    half, off = c // CPH, (c % CPH) * TOKC
                    fts = []
                    for h in range(HT):
                        ps = kp_pool.tile([128, TOKC], f32, tag="ps")
                        for k in range(KT):
                            nc.tensor.matmul(
                                ps[:],
                                wk_bf[:, k, h * 128:(h + 1) * 128],
                                kt_tiles[(half, k)][:, off:off + TOKC],
                                start=(k == 0), stop=(k == KT - 1),
                            )
                        ft = feat_pool.tile([128, TOKC], bf16, tag="ft")
                        nc.scalar.activation(ft[:], ps[:], Act.Tanh,
                                             bias=qbias_sb[:, h, b:b + 1])
                        fts.append(ft)
                    sc = sc_pool.tile([128, TOKC], f32, tag="sc")
                    for h in range(HT):
                        nc.tensor.matmul(sc[r:r + 1, :], wv_bf[:, h:h + 1],
                                         fts[h][:], start=(h == 0),
                                         stop=(h == HT - 1),
                                         tile_position=(0, r))
                    # exp straight off PSUM; partial sum fused into the same op
                    cs = c * TOKC
                    nc.scalar.activation(esc_sb[r:r + 1, cs:cs + TOKC],
                                         sc[r:r + 1, :], Act.Exp,
                                         accum_out=psum_sb[r:r + 1, c:c + 1])
                    nc.vector.tensor_mul(esc_sb[r:r + 1, cs:cs + TOKC],
                                         esc_sb[r:r + 1, cs:cs + TOKC],
                                         vals_sb[r:r + 1, cs:cs + TOKC])

                # softmax denominator + (e * 1/sum) * values, fused on GpSimd
                nc.vector.reduce_sum(ssum_sb[r:r + 1, :], psum_sb[r:r + 1, :],
                                     axis=AX)
                nc.vector.reciprocal(recip_sb[r:r + 1, :], ssum_sb[r:r + 1, :])
                nc.vector.tensor_scalar_mul(esc_sb[r:r + 1, :],
                                            esc_sb[r:r + 1, :],
                                            recip_sb[r:r + 1, :])
                nc.sync.dma_start(out_ext[b:b + 1, :], esc_sb[r:r + 1, :])

    nc.compile()
    return nc


def shard_inputs(queries, keys, values, W_q, W_k, w_v):
    queries = np.asarray(queries, np.float32)
    keys = np.asarray(keys, np.float32)
    values = np.asarray(values, np.float32)
    W_q = np.asarray(W_q, np.float32)
    W_k = np.asarray(W_k, np.float32)
    w_v = np.asarray(w_v, np.float32)

    wv2 = np.ascontiguousarray(w_v.reshape(HT, 128).T)  # [128, HT]
    in_maps = []
    for i in range(N_CORES):
        b0, b1 = i * B_LOC, (i + 1) * B_LOC
        in_maps.append({
            "keysT": np.ascontiguousarray(keys[b0:b1].transpose(0, 2, 1)),
            "queriesT": np.ascontiguousarray(queries[b0:b1, 0, :].T),
            "vals": np.ascontiguousarray(values[b0:b1, :, 0]),
            "wk": W_k, "wq": W_q, "wv": wv2,
        })
    return in_maps


_NC_CACHE = {}


def run(in_maps, trace=False, tmpdir=None):
    from concourse.bass_utils import run_bass_kernel_spmd

    _install_profile_hook()
    if trace:
        # no artifact bucket inside the container; keep traces local
        import concourse.bass_utils as bu
        bu.upload_artifacts = lambda d: "local://" + d
    if "nc" not in _NC_CACHE:
        _NC_CACHE["nc"] = build_nc()
    nc = _NC_CACHE["nc"]
    return run_bass_kernel_spmd(nc, in_maps, core_ids=list(range(N_CORES)),
                                trace=trace, tmpdir=tmpdir)


def kernel(queries, keys, values, W_q, W_k, w_v):
    in_maps = shard_inputs(queries, keys, values, W_q, W_k, w_v)
    res = run(in_maps)
    return np.concatenate([res.results[i]["out"] for i in range(N_CORES)], axis=0)
